# revision 38
# baseline (speedup 1.0000x reference)
import os
import sys

for _p in ("/opt/trn_rl_repo", "/root/.axon_site/_ro/trn_rl_repo"):
    if os.path.isdir(_p) and _p not in sys.path:
        sys.path.insert(0, _p)

import numpy as np
import concourse.bacc as bacc
import concourse.mybir as mybir
import concourse.tile as tile
from concourse import bass_utils

B, N, T, F = 8, 128, 2048, 32
L, H = 5, 64

FP32 = mybir.dt.float32
FP16 = mybir.dt.float16

TT = 256          # t-steps per x tile
HALO = 4          # max_lag - 1
CHUNK = 16        # t-steps per output chunk
NTILES = T // TT  # 8
NCHUNKS = TT // CHUNK  # 16 per tile

X_TILE_FREE = (TT + HALO) * F  # 8320
Y_CHUNK_FREE = CHUNK * H       # 1024

_CACHE = {}
LAST_RESULTS = None


def _build_nc():
    nc = bacc.Bacc("TRN2", target_bir_lowering=False, debug=False)
    x_d = nc.dram_tensor("x", (N, T * F), FP16, kind="ExternalInput").ap()
    at_d = nc.dram_tensor("at", (N, L * N), FP16, kind="ExternalInput").ap()
    wblk_d = nc.dram_tensor("wblk", (128, 256), FP16, kind="ExternalInput").ap()
    bvec_d = nc.dram_tensor("bvec", (128, 1), FP32, kind="ExternalInput").ap()
    y_d = nc.dram_tensor("y", (N, T * H), FP16, kind="ExternalOutput").ap()

    gelu = mybir.ActivationFunctionType.Gelu

    with tile.TileContext(nc) as tc:
        with (
            tc.tile_pool(name="consts", bufs=1) as consts,
            tc.tile_pool(name="xpool", bufs=2) as xpool,
            tc.tile_pool(name="aggpool", bufs=3) as aggpool,
            tc.tile_pool(name="btpool", bufs=3) as btpool,
            tc.tile_pool(name="ypool", bufs=8) as ypool,
            tc.tile_pool(name="pagg", bufs=4, space="PSUM") as pagg,
            tc.tile_pool(name="py", bufs=2, space="PSUM") as py,
        ):
            x_tiles = {}
            bt_of = {}
            pair_sb = [None]
            TOTAL = NTILES * NCHUNKS

            # DMA issue on the Sync queue costs ~600ns each, so order the
            # issues by criticality: at (stage-1 weights), then chunk 0's x
            # columns, then everything else.  This lets the first real
            # matmul chain straight onto the warmups with no PE idle gap
            # (an idle gap voids a HAM activity epoch and delays the 2.4
            # GHz un-throttle by a full 3.4us window).
            at_sb = consts.tile((N, L * N), FP16)
            w_sb = consts.tile((128, 256), FP16)
            bvec_sb = consts.tile((128, 1), FP32)
            nc.sync.dma_start(out=at_sb, in_=at_d)

            x_tile0 = xpool.tile((N, X_TILE_FREE), FP16, name="x_tile")
            x_tiles[0] = x_tile0
            nc.gpsimd.memset(x_tile0[:, 0 : HALO * F], 0)
            SL0 = TT * F // 16
            for s in range(3):
                nc.sync.dma_start(
                    out=x_tile0[:, HALO * F + s * SL0 : HALO * F + (s + 1) * SL0],
                    in_=x_d[:, s * SL0 : (s + 1) * SL0],
                )

            nc.sync.dma_start(out=w_sb, in_=wblk_d)
            nc.sync.dma_start(out=bvec_sb, in_=bvec_d)

            # PE warmup: run dummy matmuls on a zeroed tile while the first
            # x DMAs are in flight, so the HAM clock-gate reaches 8/8 (2.4
            # GHz) before the first real matmul issues.
            warm_sb = consts.tile((128, 512), FP16)
            nc.vector.memset(warm_sb, 0)
            psum_warm = pagg.tile((128, 512), FP32, name="psum_agg")
            for _ in range(10):
                nc.tensor.matmul(
                    psum_warm[:, 0:256], warm_sb[:, 0:128], warm_sb[:, 0:256],
                    start=True, stop=True,
                )

            for s in range(3, 16):
                nc.sync.dma_start(
                    out=x_tile0[:, HALO * F + s * SL0 : HALO * F + (s + 1) * SL0],
                    in_=x_d[:, s * SL0 : (s + 1) * SL0],
                )

            def emit_xload(ti):
                x_tile = xpool.tile((N, X_TILE_FREE), FP16)
                x_tiles[ti] = x_tile
                t0 = ti * TT
                src = x_d[:, (t0 - HALO) * F : (t0 + TT) * F]
                sl = X_TILE_FREE // 8
                for s in range(8):
                    nc.sync.dma_start(
                        out=x_tile[:, s * sl : (s + 1) * sl],
                        in_=src[:, s * sl : (s + 1) * sl],
                    )

            def emit_s1_last(g):
                # final chunk: run the whole pipeline in two 8-t halves (on
                # separate PSUM banks) so the end-of-kernel drain chain is
                # half as deep
                ti, c = divmod(g, NCHUNKS)
                x_tile = x_tiles[ti]
                bts = []
                for hh in range(2):
                    psum_agg = pagg.tile((N, 512), FP32, name="psum_agg")
                    for lag in range(L):
                        off = (HALO + CHUNK * c + 8 * hh - lag) * F
                        nc.tensor.matmul(
                            psum_agg[:, 0:256],
                            at_sb[:, lag * N : (lag + 1) * N],
                            x_tile[:, off : off + 256],
                            start=(lag == 0),
                            stop=(lag == L - 1),
                        )
                    sbuf_agg = aggpool.tile((N, 512), FP16, name="sbuf_agg")
                    nc.vector.tensor_copy(sbuf_agg[:, 0:256], psum_agg[:, 0:256])
                    sbuf_bt = btpool.tile((N, 512), FP16, name="sbuf_bt")
                    nc.vector.transpose(sbuf_bt[:, 0:256], sbuf_agg[:, 0:256])
                    bts.append(sbuf_bt)
                bt_of[g] = bts

            def emit_s1(g):
                if g == TOTAL - 1:
                    emit_s1_last(g)
                    return
                ti, c = divmod(g, NCHUNKS)
                x_tile = x_tiles[ti]
                psum_agg = pagg.tile((N, 512), FP32)
                for lag in range(L):
                    off = (HALO + CHUNK * c - lag) * F
                    nc.tensor.matmul(
                        psum_agg,
                        at_sb[:, lag * N : (lag + 1) * N],
                        x_tile[:, off : off + 512],
                        start=(lag == 0),
                        stop=(lag == L - 1),
                    )
                # 32x32 block transpose on DVE: since F == 32, each 32-col
                # block is one t-step, so this puts f on partitions within
                # each 32-partition group:  bt[32a+f, 32t+i] = agg[32a+i, 32t+f]
                # (StreamTranspose can't cast, so cast fp32->fp16 first)
                sbuf_agg = aggpool.tile((N, 512), FP16)
                nc.vector.tensor_copy(sbuf_agg, psum_agg)
                sbuf_bt = btpool.tile((N, 512), FP16)
                nc.vector.transpose(sbuf_bt, sbuf_agg)
                bt_of[g] = sbuf_bt

            def emit_s2(g):
                bt_val = bt_of.pop(g)
                psum_y = py.tile((N, Y_CHUNK_FREE), FP32)
                if g == TOTAL - 1:
                    for hh in range(2):
                        for r in range(2):
                            nc.tensor.matmul(
                                psum_y[:, r * 512 + hh * 256 :
                                       r * 512 + hh * 256 + 256],
                                w_sb[:, r * 128 : (r + 1) * 128],
                                bt_val[hh][:, 0:256],
                                start=True,
                                stop=True,
                            )
                else:
                    sbuf_bt = bt_val
                    for r in range(2):
                        nc.tensor.matmul(
                            psum_y[:, r * 512 : (r + 1) * 512],
                            w_sb[:, r * 128 : (r + 1) * 128],
                            sbuf_bt,
                            start=True,
                            stop=True,
                        )
                if g == TOTAL - 1:
                    # split the final chunk into its two matmul halves so the
                    # first half's activation/store overlaps the second matmul
                    sbuf_y = ypool.tile((N, Y_CHUNK_FREE), FP16)
                    for q in range(2):
                        sl = slice(q * 512, (q + 1) * 512)
                        nc.scalar.activation(
                            sbuf_y[:, sl], psum_y[:, sl], func=gelu, bias=bvec_sb
                        )
                        nc.sync.dma_start(
                            out=y_d[:, g * Y_CHUNK_FREE + q * 512 :
                                    g * Y_CHUNK_FREE + (q + 1) * 512],
                            in_=sbuf_y[:, sl],
                        )
                elif g == TOTAL - 2:
                    sbuf_y = ypool.tile((N, Y_CHUNK_FREE), FP16)
                    nc.scalar.activation(sbuf_y, psum_y, func=gelu, bias=bvec_sb)
                    nc.sync.dma_start(
                        out=y_d[:, g * Y_CHUNK_FREE : (g + 1) * Y_CHUNK_FREE],
                        in_=sbuf_y,
                    )
                else:
                    # coalesce stores in pairs of chunks: one 512 KB DMA per
                    # two chunks halves the store-issue + semaphore traffic
                    if g % 2 == 0:
                        pair_sb[0] = ypool.tile(
                            (N, 2 * Y_CHUNK_FREE), FP16, name="ypair"
                        )
                    sb = pair_sb[0]
                    half = slice((g % 2) * Y_CHUNK_FREE, (g % 2 + 1) * Y_CHUNK_FREE)
                    nc.scalar.activation(sb[:, half], psum_y, func=gelu, bias=bvec_sb)
                    if g % 2 == 1:
                        nc.sync.dma_start(
                            out=y_d[:, (g - 1) * Y_CHUNK_FREE : (g + 1) * Y_CHUNK_FREE],
                            in_=sb,
                        )

            for g in range(TOTAL + 2):
                if g < TOTAL:
                    ti, c = divmod(g, NCHUNKS)
                    emit_s1(g)
                    if c == 0 and ti + 1 < NTILES:
                        emit_xload(ti + 1)
                if g >= 2:
                    emit_s2(g - 2)
    nc.compile()
    return nc


def kernel(x, A_list, W, b):
    global LAST_RESULTS
    x = np.asarray(x, np.float32)
    A_list = np.asarray(A_list, np.float32)
    W = np.asarray(W, np.float32)
    b = np.asarray(b, np.float32)

    if "nc" not in _CACHE:
        _CACHE["nc"] = _build_nc()
    nc = _CACHE["nc"]

    # stage-2 weight packing: w[32a + f, 128r + 64da + h] = W[h, f]
    # where a = 2r + da  (a = high 2 bits of the node index i)
    wblk = np.zeros((128, 256), np.float16)
    wt = W.T.astype(np.float16)  # [32 f, 64 h]
    for a in range(4):
        r, da = divmod(a, 2)
        wblk[a * F : (a + 1) * F, r * 128 + da * H : r * 128 + (da + 1) * H] = wt
    bvec = np.ascontiguousarray(np.tile(b, 2)[:, None].astype(np.float32))

    in_maps = []
    for c in range(B):
        in_maps.append(
            {
                "x": x[c].reshape(N, T * F).astype(np.float16),
                "at": np.ascontiguousarray(
                    A_list[c].transpose(2, 0, 1).reshape(N, L * N)
                ).astype(np.float16),
                "wblk": wblk,
                "bvec": bvec,
            }
        )

    trace = bool(os.environ.get("KERNEL_TRACE"))
    res = bass_utils.run_bass_kernel_spmd(
        nc, in_maps, core_ids=list(range(B)), trace=trace
    )
    LAST_RESULTS = res
    outs = []
    for c in range(B):
        arr = np.asarray(res.results[c]["y"])
        # row p = 64*da + h ; col = 1024*g + 512*r + 32*tl + ip
        # y[i, t, h] with i = 64r + 32da + ip, t = 16g + tl
        arr6 = arr.reshape(2, 64, 128, 2, 16, 32)
        yb = (
            np.transpose(arr6, (3, 0, 5, 2, 4, 1))
            .reshape(N, T, H)
            .astype(np.float32)
        )
        outs.append(yb)
    return np.stack(outs)


# revision 40
# speedup vs baseline: 1.0063x; 1.0063x over previous
import os
import sys

for _p in ("/opt/trn_rl_repo", "/root/.axon_site/_ro/trn_rl_repo"):
    if os.path.isdir(_p) and _p not in sys.path:
        sys.path.insert(0, _p)

import numpy as np
import concourse.bacc as bacc
import concourse.mybir as mybir
import concourse.tile as tile
from concourse import bass_utils

B, N, T, F = 8, 128, 2048, 32
L, H = 5, 64

FP32 = mybir.dt.float32
FP16 = mybir.dt.float16

TT = 256          # t-steps per x tile
HALO = 4          # max_lag - 1
CHUNK = 16        # t-steps per output chunk
NTILES = T // TT  # 8
NCHUNKS = TT // CHUNK  # 16 per tile

X_TILE_FREE = (TT + HALO) * F  # 8320
Y_CHUNK_FREE = CHUNK * H       # 1024

_CACHE = {}
LAST_RESULTS = None


def _build_nc():
    nc = bacc.Bacc("TRN2", target_bir_lowering=False, debug=False)
    x_d = nc.dram_tensor("x", (N, T * F), FP16, kind="ExternalInput").ap()
    at_d = nc.dram_tensor("at", (N, L * N), FP16, kind="ExternalInput").ap()
    wblk_d = nc.dram_tensor("wblk", (128, 256), FP16, kind="ExternalInput").ap()
    bvec_d = nc.dram_tensor("bvec", (128, 1), FP32, kind="ExternalInput").ap()
    y_d = nc.dram_tensor("y", (N, T * H), FP16, kind="ExternalOutput").ap()

    gelu = mybir.ActivationFunctionType.Gelu

    with tile.TileContext(nc) as tc:
        with (
            tc.tile_pool(name="consts", bufs=1) as consts,
            tc.tile_pool(name="xpool", bufs=2) as xpool,
            tc.tile_pool(name="aggpool", bufs=3) as aggpool,
            tc.tile_pool(name="btpool", bufs=3) as btpool,
            tc.tile_pool(name="ypool", bufs=8) as ypool,
            tc.tile_pool(name="pagg", bufs=4, space="PSUM") as pagg,
            tc.tile_pool(name="py", bufs=2, space="PSUM") as py,
        ):
            x_tiles = {}
            bt_of = {}
            pair_sb = [None]
            TOTAL = NTILES * NCHUNKS

            # DMA issue on the Sync queue costs ~600ns each, so order the
            # issues by criticality: at (stage-1 weights), then chunk 0's x
            # columns, then everything else.  This lets the first real
            # matmul chain straight onto the warmups with no PE idle gap
            # (an idle gap voids a HAM activity epoch and delays the 2.4
            # GHz un-throttle by a full 3.4us window).
            at_sb = consts.tile((N, L * N), FP16)
            w_sb = consts.tile((128, 256), FP16)
            bvec_sb = consts.tile((128, 1), FP32)
            nc.sync.dma_start(out=at_sb, in_=at_d)

            x_tile0 = xpool.tile((N, X_TILE_FREE), FP16, name="x_tile")
            x_tiles[0] = x_tile0
            nc.gpsimd.memset(x_tile0[:, 0 : HALO * F], 0)
            SL0 = TT * F // 16
            for s in range(3):
                nc.sync.dma_start(
                    out=x_tile0[:, HALO * F + s * SL0 : HALO * F + (s + 1) * SL0],
                    in_=x_d[:, s * SL0 : (s + 1) * SL0],
                )

            nc.sync.dma_start(out=w_sb, in_=wblk_d)
            nc.sync.dma_start(out=bvec_sb, in_=bvec_d)

            # PE warmup: run dummy matmuls on a zeroed tile while the first
            # x DMAs are in flight, so the HAM clock-gate reaches 8/8 (2.4
            # GHz) before the first real matmul issues.
            warm_sb = consts.tile((128, 512), FP16)
            nc.vector.memset(warm_sb, 0)
            psum_warm = pagg.tile((128, 512), FP32, name="psum_agg")
            for _ in range(10):
                nc.tensor.matmul(
                    psum_warm[:, 0:256], warm_sb[:, 0:128], warm_sb[:, 0:256],
                    start=True, stop=True,
                )

            for s in range(3, 16):
                nc.sync.dma_start(
                    out=x_tile0[:, HALO * F + s * SL0 : HALO * F + (s + 1) * SL0],
                    in_=x_d[:, s * SL0 : (s + 1) * SL0],
                )

            def emit_xload(ti):
                x_tile = xpool.tile((N, X_TILE_FREE), FP16)
                x_tiles[ti] = x_tile
                t0 = ti * TT
                src = x_d[:, (t0 - HALO) * F : (t0 + TT) * F]
                sl = X_TILE_FREE // 8
                for s in range(8):
                    nc.sync.dma_start(
                        out=x_tile[:, s * sl : (s + 1) * sl],
                        in_=src[:, s * sl : (s + 1) * sl],
                    )

            def emit_s1_last(g):
                # final chunk: run the whole pipeline in two 8-t halves (on
                # separate PSUM banks) so the end-of-kernel drain chain is
                # half as deep
                ti, c = divmod(g, NCHUNKS)
                x_tile = x_tiles[ti]
                bts = []
                for hh in range(2):
                    psum_agg = pagg.tile((N, 512), FP32, name="psum_agg")
                    for lag in range(L):
                        off = (HALO + CHUNK * c + 8 * hh - lag) * F
                        nc.tensor.matmul(
                            psum_agg[:, 0:256],
                            at_sb[:, lag * N : (lag + 1) * N],
                            x_tile[:, off : off + 256],
                            start=(lag == 0),
                            stop=(lag == L - 1),
                        )
                    sbuf_agg = aggpool.tile((N, 512), FP16, name="sbuf_agg")
                    nc.vector.tensor_copy(sbuf_agg[:, 0:256], psum_agg[:, 0:256])
                    sbuf_bt = btpool.tile((N, 512), FP16, name="sbuf_bt")
                    nc.vector.transpose(sbuf_bt[:, 0:256], sbuf_agg[:, 0:256])
                    bts.append(sbuf_bt)
                bt_of[g] = bts

            def emit_s1(g):
                if g == TOTAL - 1:
                    emit_s1_last(g)
                    return
                ti, c = divmod(g, NCHUNKS)
                x_tile = x_tiles[ti]
                psum_agg = pagg.tile((N, 512), FP32)
                for lag in range(L):
                    off = (HALO + CHUNK * c - lag) * F
                    nc.tensor.matmul(
                        psum_agg,
                        at_sb[:, lag * N : (lag + 1) * N],
                        x_tile[:, off : off + 512],
                        start=(lag == 0),
                        stop=(lag == L - 1),
                    )
                # 32x32 block transpose on DVE: since F == 32, each 32-col
                # block is one t-step, so this puts f on partitions within
                # each 32-partition group:  bt[32a+f, 32t+i] = agg[32a+i, 32t+f]
                # (StreamTranspose can't cast, so cast fp32->fp16 first)
                sbuf_agg = aggpool.tile((N, 512), FP16)
                nc.vector.tensor_copy(sbuf_agg, psum_agg)
                sbuf_bt = btpool.tile((N, 512), FP16)
                nc.vector.transpose(sbuf_bt, sbuf_agg)
                bt_of[g] = sbuf_bt

            def emit_s2(g):
                bt_val = bt_of.pop(g)
                psum_y = py.tile((N, Y_CHUNK_FREE), FP32)
                if g == TOTAL - 1:
                    for hh in range(2):
                        for r in range(2):
                            nc.tensor.matmul(
                                psum_y[:, r * 512 + hh * 256 :
                                       r * 512 + hh * 256 + 256],
                                w_sb[:, r * 128 : (r + 1) * 128],
                                bt_val[hh][:, 0:256],
                                start=True,
                                stop=True,
                            )
                else:
                    sbuf_bt = bt_val
                    for r in range(2):
                        nc.tensor.matmul(
                            psum_y[:, r * 512 : (r + 1) * 512],
                            w_sb[:, r * 128 : (r + 1) * 128],
                            sbuf_bt,
                            start=True,
                            stop=True,
                        )
                if g == TOTAL - 1:
                    # split the final chunk into its two matmul halves so the
                    # first half's activation/store overlaps the second matmul
                    sbuf_y = ypool.tile((N, Y_CHUNK_FREE), FP16)
                    for q in range(2):
                        sl = slice(q * 512, (q + 1) * 512)
                        nc.scalar.activation(
                            sbuf_y[:, sl], psum_y[:, sl], func=gelu, bias=bvec_sb
                        )
                        nc.sync.dma_start(
                            out=y_d[:, g * Y_CHUNK_FREE + q * 512 :
                                    g * Y_CHUNK_FREE + (q + 1) * 512],
                            in_=sbuf_y[:, sl],
                        )
                elif g == TOTAL - 2:
                    sbuf_y = ypool.tile((N, Y_CHUNK_FREE), FP16)
                    nc.scalar.activation(sbuf_y, psum_y, func=gelu, bias=bvec_sb)
                    nc.sync.dma_start(
                        out=y_d[:, g * Y_CHUNK_FREE : (g + 1) * Y_CHUNK_FREE],
                        in_=sbuf_y,
                    )
                else:
                    # coalesce stores in pairs of chunks: one 512 KB DMA per
                    # two chunks halves the store-issue + semaphore traffic
                    if g % 2 == 0:
                        pair_sb[0] = ypool.tile(
                            (N, 2 * Y_CHUNK_FREE), FP16, name="ypair"
                        )
                    sb = pair_sb[0]
                    half = slice((g % 2) * Y_CHUNK_FREE, (g % 2 + 1) * Y_CHUNK_FREE)
                    nc.scalar.activation(sb[:, half], psum_y, func=gelu, bias=bvec_sb)
                    if g % 2 == 1:
                        nc.sync.dma_start(
                            out=y_d[:, (g - 1) * Y_CHUNK_FREE : (g + 1) * Y_CHUNK_FREE],
                            in_=sb,
                        )

            for g in range(TOTAL + 2):
                if g < TOTAL:
                    ti, c = divmod(g, NCHUNKS)
                    emit_s1(g)
                    if c == 0 and ti + 1 < NTILES:
                        emit_xload(ti + 1)
                if g >= 2:
                    emit_s2(g - 2)
    nc.compile()
    return nc


def kernel(x, A_list, W, b):
    global LAST_RESULTS
    x = np.asarray(x, np.float32)
    A_list = np.asarray(A_list, np.float32)
    W = np.asarray(W, np.float32)
    b = np.asarray(b, np.float32)

    if "nc" not in _CACHE:
        _CACHE["nc"] = _build_nc()
    nc = _CACHE["nc"]

    # stage-2 weight packing: w[32a + f, 128r + 64da + h] = W[h, f]
    # where a = 2r + da  (a = high 2 bits of the node index i)
    wblk = np.zeros((128, 256), np.float16)
    wt = W.T.astype(np.float16)  # [32 f, 64 h]
    for a in range(4):
        r, da = divmod(a, 2)
        wblk[a * F : (a + 1) * F, r * 128 + da * H : r * 128 + (da + 1) * H] = wt
    bvec = np.ascontiguousarray(np.tile(b, 2)[:, None].astype(np.float32))

    in_maps = []
    for c in range(B):
        in_maps.append(
            {
                "x": x[c].reshape(N, T * F).astype(np.float16),
                "at": np.ascontiguousarray(
                    A_list[c].transpose(2, 0, 1).reshape(N, L * N)
                ).astype(np.float16),
                "wblk": wblk,
                "bvec": bvec,
            }
        )

    trace = bool(os.environ.get("KERNEL_TRACE"))
    res = bass_utils.run_bass_kernel_spmd(
        nc, in_maps, core_ids=list(range(B)), trace=trace
    )
    LAST_RESULTS = res
    outs = []
    for c in range(B):
        arr = np.asarray(res.results[c]["y"])
        # row p = 64*da + h ; col = 1024*g + 512*r + 32*tl + ip
        # y[i, t, h] with i = 64r + 32da + ip, t = 16g + tl
        arr6 = arr.reshape(2, 64, 128, 2, 16, 32)
        yb = (
            np.transpose(arr6, (3, 0, 5, 2, 4, 1))
            .reshape(N, T, H)
            .astype(np.float32)
        )
        outs.append(yb)
    return np.stack(outs)
